# revision 1
# baseline (speedup 1.0000x reference)
"""Trainium2 Bass kernel for nn_ClassicMHA (dense transformer block, linear attention).

Sharding: data-parallel over batch B=8 across the 8 NeuronCores (one batch
element per core, no collectives).

Per-core dataflow (channels-major (C, N) everywhere, N=4096 tokens):
  pass 1: K,V token-major projections per 512-token tile; dot_h = K_h^T V_h
          accumulated in 4 persistent PSUM banks (head pairs, block-diagonal).
  softmax on the 8 (64,64) head dots -> block-diag attn lhsT tiles.
  pass 2: QT -> yT = attn^T QT -> mha = Wo^T yT (Wo,bo pre-doubled on host to
          fold the reference's ln(y+y)) -> LN1 -> z1 = relu(W1^T ln1 + b1)
          -> z2 = W2^T z1 + b2 -> LN2(ln1 + z2) -> out.
  LayerNorm over channels (= partitions) uses ones-vector colsum matmuls for
  stats and K=1 outer-product matmuls to broadcast per-token scalars.

All matmuls run in float32r (TF32-ish, 1 cycle/row at N>=256) with fp32 PSUM
accumulation; fp32r operands are produced by DMA-bitcast or fp32r-typed
engine outputs to satisfy the BIR verifier's rounding rule.
"""

import contextlib
import ctypes
import os
import sys
import types

import numpy as np

# ---------------------------------------------------------------------------
# environment setup: jax persistent compile cache + ntff profile hook shim
# ---------------------------------------------------------------------------

def _setup_env():
    try:
        import jax
        cache_dir = os.environ.get("BASS_JAX_CACHE", "/root/jaxcache")
        os.makedirs(cache_dir, exist_ok=True)
        jax.config.update("jax_compilation_cache_dir", cache_dir)
        jax.config.update("jax_persistent_cache_min_entry_size_bytes", -1)
        jax.config.update("jax_persistent_cache_min_compile_time_secs", 0)
    except Exception:
        pass

    try:
        from antenv.axon_hooks import get_axon_ntff_profile_hook  # noqa: F401
        return
    except ImportError:
        pass
    mod = types.ModuleType("antenv.axon_hooks")
    _holder = {}
    mod.set_axon_ntff_profile_hook = lambda h: _holder.__setitem__("h", h)
    mod.get_axon_ntff_profile_hook = lambda: _holder.get("h")
    sys.modules["antenv.axon_hooks"] = mod
    try:
        import antenv
        antenv.axon_hooks = mod
    except ImportError:
        pass
    try:
        lib = ctypes.CDLL("/opt/axon/libaxon_pjrt.so")
        if not hasattr(lib, "axon_start_nrt_profile"):
            return
        lib.axon_start_nrt_profile.argtypes = [ctypes.POINTER(ctypes.c_int64), ctypes.c_size_t]
        lib.axon_start_nrt_profile.restype = ctypes.c_int64
        lib.axon_stop_nrt_profile.argtypes = [ctypes.c_char_p]
        lib.axon_stop_nrt_profile.restype = ctypes.c_int64

        @contextlib.contextmanager
        def _hook(output_dir, device_ids):
            import jax
            jax.devices()
            if device_ids:
                ids = (ctypes.c_int64 * len(device_ids))(*device_ids)
                rc = lib.axon_start_nrt_profile(ids, len(device_ids))
            else:
                rc = lib.axon_start_nrt_profile(None, 0)
            if rc != 0:
                raise RuntimeError(f"axon_start_nrt_profile rc={rc}")
            try:
                yield
            finally:
                n = lib.axon_stop_nrt_profile(str(output_dir).encode())
                print(f"profile: {n} file(s) -> {output_dir}", file=sys.stderr)

        mod.set_axon_ntff_profile_hook(_hook)
    except Exception:
        pass


_setup_env()

import concourse.bass as bass  # noqa: E402
import concourse.tile as tile  # noqa: E402
from concourse import bacc, mybir  # noqa: E402
from concourse.bass_utils import run_bass_kernel_spmd  # noqa: E402

f32 = mybir.dt.float32
f32r = mybir.dt.float32r
AF = mybir.ActivationFunctionType
Alu = mybir.AluOpType
AX = mybir.AxisListType

B, D, N, H, HD = 8, 512, 4096, 8, 64
FF = 4 * D            # 2048
T = 512               # tokens per n-tile
NT = N // T           # 8
KD = D // 128         # 4 k-tiles over model dim
MD = D // 128         # 4 m-tiles over model dim
FM = FF // 128        # 16 m-tiles over ffn dim
PAIRS = H // 2        # 4 head pairs (2x64 channels)
EPS = 1e-5


def build_nc():
    nc = bacc.Bacc("TRN2", target_bir_lowering=False, debug=False)

    x_d = nc.dram_tensor("x", [D, N], f32, kind="ExternalInput")
    wq_d = nc.dram_tensor("wq", [D, D], f32, kind="ExternalInput")
    wk_d = nc.dram_tensor("wk", [D, D], f32, kind="ExternalInput")
    wv_d = nc.dram_tensor("wv", [D, D], f32, kind="ExternalInput")
    wo_d = nc.dram_tensor("wo", [D, D], f32, kind="ExternalInput")
    w1_d = nc.dram_tensor("w1", [D, FF], f32, kind="ExternalInput")
    w2_d = nc.dram_tensor("w2", [FF, D], f32, kind="ExternalInput")
    bq_d = nc.dram_tensor("bq", [D], f32, kind="ExternalInput")
    bk_d = nc.dram_tensor("bk", [D], f32, kind="ExternalInput")
    bv_d = nc.dram_tensor("bv", [D], f32, kind="ExternalInput")
    bo_d = nc.dram_tensor("bo", [D], f32, kind="ExternalInput")
    b1_d = nc.dram_tensor("b1", [FF], f32, kind="ExternalInput")
    b2_d = nc.dram_tensor("b2", [D], f32, kind="ExternalInput")
    g1_d = nc.dram_tensor("g1", [D], f32, kind="ExternalInput")
    be1_d = nc.dram_tensor("be1", [D], f32, kind="ExternalInput")
    g2_d = nc.dram_tensor("g2", [D], f32, kind="ExternalInput")
    be2_d = nc.dram_tensor("be2", [D], f32, kind="ExternalInput")
    out_d = nc.dram_tensor("out", [D, N], f32, kind="ExternalOutput")

    col = lambda d: d.ap().rearrange("(p o) -> p o", o=1)
    row = lambda d: d.ap().rearrange("(o f) -> o f", o=1)

    with tile.TileContext(nc) as tc, contextlib.ExitStack() as top:
        wp = top.enter_context(tc.tile_pool(name="wts", bufs=1))
        xp = top.enter_context(tc.tile_pool(name="xp", bufs=8))
        rows = top.enter_context(tc.tile_pool(name="rows", bufs=4))
        smalls = top.enter_context(tc.tile_pool(name="smalls", bufs=2))

        def w_tile(dram, k, ncols, tag, pool=None):
            t_ = (pool or wp).tile([128, ncols], f32r, tag=f"{tag}{k}",
                                   bufs=1 if pool else None,
                                   name=f"{tag}{k}")
            nc.sync.dma_start(
                t_[:], dram.ap()[k * 128:(k + 1) * 128, :].bitcast(f32r))
            return t_

        def load_cols(dram, nm, tag):
            ts = []
            for m in range(nm):
                t_ = wp.tile([128, 1], f32, tag=f"{tag}{m}", name=f"{tag}{m}")
                nc.sync.dma_start(t_[:], col(dram)[m * 128:(m + 1) * 128, :])
                ts.append(t_)
            return ts

        # --- startup-critical loads only: bk/bv rows, consts, WK, WV ---
        bk_r = wp.tile([1, D], f32r, tag="bkr")
        nc.sync.dma_start(bk_r[:], row(bk_d).bitcast(f32r))
        bv_r = wp.tile([1, D], f32r, tag="bvr")
        nc.sync.dma_start(bv_r[:], row(bv_d).bitcast(f32r))
        ones_c32 = wp.tile([128, 1], f32, tag="onc32")
        nc.vector.memset(ones_c32[:], 1.0)
        ones_c = wp.tile([128, 1], f32r, tag="onc")
        nc.vector.tensor_copy(ones_c[:], ones_c32[:])
        ones_r32 = wp.tile([1, 128], f32, tag="onr32")
        nc.vector.memset(ones_r32[:], 1.0)
        ones_r = wp.tile([1, 128], f32r, tag="onr")
        nc.vector.tensor_copy(ones_r[:], ones_r32[:])
        eps_c = wp.tile([1, 1], f32, tag="epsc")
        nc.vector.memset(eps_c[:], EPS)

        WK, WV = [None] * KD, [None] * KD
        BKB = BVB = None
        WNO = [wp.tile([128, D], f32r, tag=f"wno{p}", name=f"wno{p}")
               for p in range(PAIRS)]
        WQNO = [wp.tile([128, D], f32r, tag=f"wqno{k}", name=f"wqno{k}")
                for k in range(KD)]
        WQT = [wp.tile([128, D], f32r, tag=f"wqt{dm}", name=f"wqt{dm}")
               for dm in range(MD)]
        MHB = [wp.tile([128, 1], f32, tag=f"mhb{m}", name=f"mhb{m}")
               for m in range(MD)]
        BQR = []
        for k in range(KD):
            t_ = wp.tile([128, 2], f32r, tag=f"bqr{k}", name=f"bqr{k}")
            for c in range(2):
                nc.sync.dma_start(t_[:, c:c + 1],
                                  col(bq_d)[k * 128:(k + 1) * 128,
                                            :].bitcast(f32r))
            BQR.append(t_)
        BD = [wp.tile([128, 128], f32r, tag=f"bd{p}", name=f"bd{p}")
              for p in range(PAIRS)]
        ident = wp.tile([128, 128], f32r, tag="idr")
        WQ, WO, W1 = [None] * KD, [None] * KD, [None] * KD
        COLS = {}
        P1POOL = [None]
        P1PS = [None]

        def x_load(t):
            ts = []
            for k in range(KD):
                x_t = xp.tile([128, T], f32r, tag="x", name=f"x_{t}_{k}")
                nc.sync.dma_start(
                    x_t[:],
                    x_d.ap()[k * 128:(k + 1) * 128,
                             t * T:(t + 1) * T].bitcast(f32r))
                ts.append(x_t)
            return ts

        # deferred weight loads, spread across pass-1 iterations so they
        # never delay the x prefetch stream
        def deferred_loads(t):
            if t == 0:
                for k in range(KD):
                    WQ[k] = w_tile(wq_d, k, D, "wq", pool=P1POOL[0])
            elif t == 1:
                for k in range(KD):
                    WO[k] = w_tile(wo_d, k, D, "wo")
            elif t == 2:
                W1[0] = w_tile(w1_d, 0, FF, "w1")
                W1[1] = w_tile(w1_d, 1, FF, "w1")
            elif t == 3:
                W1[2] = w_tile(w1_d, 2, FF, "w1")
                W1[3] = w_tile(w1_d, 3, FF, "w1")
            elif t == 4:
                COLS["bq"] = load_cols(bq_d, MD, "bq")
                COLS["bo"] = load_cols(bo_d, MD, "bo")
                COLS["b1"] = load_cols(b1_d, FM, "b1")
                COLS["b2"] = load_cols(b2_d, MD, "b2")
            elif t == 5:
                COLS["g1"] = load_cols(g1_d, MD, "g1")
                COLS["be1"] = load_cols(be1_d, MD, "be1")
                COLS["g2"] = load_cols(g2_d, MD, "g2")
                COLS["be2"] = load_cols(be2_d, MD, "be2")
            if 2 <= t <= 5:
                k = t - 2
                for dm in range(MD):
                    tp = P1PS[0].tile([128, 128], f32r, tag="kvps",
                                      name=f"wqtp_{k}_{dm}")
                    nc.tensor.transpose(
                        tp[:], WQ[k][:, dm * 128:(dm + 1) * 128], ident[:])
                    nc.vector.tensor_copy(
                        WQT[dm][:, k * 128:(k + 1) * 128], tp[:])

        # =============================== pass 1 ===============================
        with tc.tile_pool(name="kv", bufs=5) as kvp, \
             tc.tile_pool(name="dps", bufs=1, space="PSUM") as dps, \
             tc.tile_pool(name="kps", bufs=4, space="PSUM") as kps:

            P1POOL[0] = kvp
            P1PS[0] = kps
            ident32 = kvp.tile([128, 128], f32, tag="id32", bufs=1,
                               name="ident32")
            from concourse.masks import make_identity
            make_identity(nc, ident32[:])
            nc.vector.tensor_copy(ident[:], ident32[:])
            BKB = kvp.tile([128, D], f32, tag="bkb", bufs=1, name="BKB")
            BVB = kvp.tile([128, D], f32, tag="bvb", bufs=1, name="BVB")
            for src_r, dst in ((bk_r, BKB), (bv_r, BVB)):
                ps = kps.tile([128, D], f32, tag="kvps", name=f"bc_{dst.name}")
                nc.tensor.matmul(ps[:], ones_r[:], src_r[:], start=True, stop=True)
                nc.vector.tensor_copy(dst[:], ps[:])

            dot = [dps.tile([128, 128], f32, tag=f"dot{p}", name=f"dot{p}")
                   for p in range(PAIRS)]
            kv_prev = None

            def emit_dot(ksb, vsb, first, last):
                for p in range(PAIRS):
                    nc.tensor.matmul(
                        dot[p][:],
                        ksb[:, p * 128:(p + 1) * 128],
                        vsb[:, p * 128:(p + 1) * 128],
                        start=first, stop=last, skip_group_check=True)

            xt1 = []
            for k in range(KD):
                x_t = xp.tile([128, T], f32r, tag="x", name=f"x_0_{k}")
                nc.sync.dma_start(
                    x_t[:], x_d.ap()[k * 128:(k + 1) * 128, 0:T].bitcast(f32r))
                xt1.append(x_t)
                WK[k] = w_tile(wk_d, k, D, "wk")
            for k in range(KD):
                WV[k] = w_tile(wv_d, k, D, "wv")
            for t in range(NT):
                xt, xt1 = xt1, (x_load(t + 1) if t + 1 < NT else None)
                deferred_loads(t)
                for st in range(T // 128):
                    xs = [x_t[:, st * 128:(st + 1) * 128] for x_t in xt]
                    kps_t = kps.tile([128, D], f32, tag="kvps", name=f"kp_{t}_{st}")
                    for k in range(KD):
                        nc.tensor.matmul(kps_t[:], xs[k], WK[k][:],
                                         start=(k == 0), stop=(k == KD - 1))
                    ksb = kvp.tile([128, D], f32r, tag="ksb", name=f"k_{t}_{st}")
                    nc.vector.tensor_tensor(ksb[:], kps_t[:], BKB[:], op=Alu.add)

                    vps_t = kps.tile([128, D], f32, tag="kvps", name=f"vp_{t}_{st}")
                    for k in range(KD):
                        nc.tensor.matmul(vps_t[:], xs[k], WV[k][:],
                                         start=(k == 0), stop=(k == KD - 1))
                    vsb = kvp.tile([128, D], f32r, tag="vsb", name=f"v_{t}_{st}")
                    nc.vector.tensor_tensor(vsb[:], vps_t[:], BVB[:], op=Alu.add)

                    if kv_prev is not None:
                        emit_dot(*kv_prev)
                    kv_prev = (ksb, vsb, t == 0 and st == 0, False)

            # ---- softmax per 64x64 head block + fold attn into Wo:
            # WNO[pair] = blockdiag(attn_pair)^T-applied Wo2 rows, so the
            # per-tile attention-apply matmul disappears entirely.
            def softmax_pair(p):
                S = smalls.tile([128, 128], f32, tag="sm_s", name=f"S{p}")
                nc.scalar.activation(S[:], dot[p][:], AF.Copy, scale=1.0 / 8.0)
                nm = smalls.tile([128, 1], f32, tag="sm_nm", name=f"nm{p}")
                E = smalls.tile([128, 128], f32, tag="sm_e", name=f"E{p}")
                se = smalls.tile([128, 1], f32, tag="sm_se", name=f"se{p}")
                ri = smalls.tile([128, 1], f32, tag="sm_ri", name=f"ri{p}")
                for h0 in (0, 64):
                    blk = slice(h0, h0 + 64)
                    nc.vector.reduce_max(nm[blk], S[blk, blk], axis=AX.X,
                                         negate=True)
                    nc.scalar.activation(E[blk, blk], S[blk, blk], AF.Exp,
                                         bias=nm[blk], accum_out=se[blk])
                nc.vector.reciprocal(ri[:], se[:])
                bd = BD[p]
                nc.vector.tensor_scalar_mul(bd[0:64, 0:64], E[0:64, 0:64], ri[0:64])
                nc.vector.tensor_scalar_mul(bd[64:128, 64:128], E[64:128, 64:128],
                                            ri[64:128])
                nc.vector.tensor_scalar_mul(bd[0:64, 64:128], S[0:64, 64:128], 0.0)
                nc.vector.tensor_scalar_mul(bd[64:128, 0:64], S[64:128, 0:64], 0.0)
            emit_dot(kv_prev[0], kv_prev[1], kv_prev[2], True)
            for p in range(PAIRS):
                softmax_pair(p)

        # =============================== pass 2 ===============================
        # =============================== pass 2 ===============================
        # Software-pipelined stages with MM-granularity interleaving: the
        # small LN stats/broadcast matmuls (M=1 / K=1) do not register as PE
        # activity for the HAM clock gate, so they are sprinkled between the
        # dense z1/z2/QT matmuls of the neighboring tile to keep the PE warm.
        with tc.tile_pool(name="mh", bufs=5) as mhp, \
             tc.tile_pool(name="scr", bufs=2) as scp, \
             tc.tile_pool(name="lnp", bufs=5) as lnp, \
             tc.tile_pool(name="z1p", bufs=3) as z1p, \
             tc.tile_pool(name="w2p", bufs=5) as w2p, \
             tc.tile_pool(name="outp", bufs=3) as outp, \
             tc.tile_pool(name="mmps", bufs=4, space="PSUM") as mmps, \
             tc.tile_pool(name="bcp", bufs=3) as bcp, \
             tc.tile_pool(name="z2ps", bufs=4, space="PSUM") as z2ps:

            ST = [dict() for _ in range(NT)]

            def ln_stats_thunks(t, s, key, ssfx):
                """Return small-op thunks computing LN stats of s[key]."""
                state = {}

                def alloc_s():
                    state["st_s"] = mmps.tile([1, T], f32, tag="mm",
                                              name=f"lns_{ssfx}")

                def s_mm(m):
                    def f():
                        if m == 0:
                            alloc_s()
                        nc.tensor.matmul(state["st_s"][:], ones_c[:],
                                         s[key][m][:], start=(m == 0),
                                         stop=(m == MD - 1))
                    return f

                def sq_op(m):
                    def f():
                        sq = scp.tile([128, T], f32r, tag="sq", bufs=2,
                                      name=f"sq_{ssfx}_{m}")
                        nc.scalar.activation(sq[:], s[key][m][:].bitcast(f32),
                                             AF.Square)
                        state[f"sq{m}"] = sq
                    return f

                def ss_mm(m):
                    def f():
                        if m == 0:
                            state["st_ss"] = mmps.tile([1, T], f32, tag="mm",
                                                       name=f"lnss_{ssfx}")
                        nc.tensor.matmul(state["st_ss"][:], ones_c[:],
                                         state[f"sq{m}"][:], start=(m == 0),
                                         stop=(m == MD - 1))
                    return f

                def rows_chain():
                    st_s, st_ss = state["st_s"], state["st_ss"]
                    r_mneg = rows.tile([1, T], f32, tag="row", name=f"mneg_{ssfx}")
                    nc.vector.tensor_scalar_mul(r_mneg[:], st_s[:], -1.0 / D)
                    r_var = rows.tile([1, T], f32, tag="row", name=f"var_{ssfx}")
                    nc.vector.tensor_scalar_mul(r_var[:], st_ss[:], 1.0 / D)
                    r_m2 = rows.tile([1, T], f32, tag="row", name=f"m2_{ssfx}")
                    nc.vector.tensor_mul(r_m2[:], r_mneg[:], r_mneg[:])
                    nc.vector.tensor_sub(r_var[:], r_var[:], r_m2[:])
                    r_rstd = rows.tile([1, T], f32, tag="rowr", bufs=5,
                                       name=f"rstd_{ssfx}")
                    nc.scalar.activation(r_rstd[:], r_var[:],
                                         AF.Abs_reciprocal_sqrt, bias=eps_c[:])
                    r_bneg = rows.tile([1, T], f32, tag="rowr", bufs=5,
                                       name=f"bneg_{ssfx}")
                    nc.vector.tensor_mul(r_bneg[:], r_mneg[:], r_rstd[:])
                    s[f"rows_{key}"] = (r_rstd, r_bneg)

                return ([s_mm(m) for m in range(MD)]
                        + [sq_op(0), sq_op(1), ss_mm(0), sq_op(2), ss_mm(1),
                           sq_op(3), ss_mm(2), ss_mm(3), rows_chain])

            def ln_norm_thunks(s, key, G, BE, out_pool, out_tag, out_dtype,
                               out_key, ssfx, dma_m=None):
                state = {}

                def r_mm():
                    r_rstd, _ = s[f"rows_{key}"]
                    R = bcp.tile([128, T], f32, tag="bcast", name=f"R_{ssfx}")
                    nc.gpsimd.partition_broadcast(R[:], r_rstd[:])
                    state["R"] = R

                def t1_ops():
                    t1s = []
                    for m in range(MD):
                        t1 = scp.tile([128, T], f32, tag="t1",
                                      name=f"t1_{ssfx}_{m}")
                        nc.vector.tensor_tensor(t1[:], s[key][m][:].bitcast(f32),
                                                state["R"][:], op=Alu.mult)
                        t1s.append(t1)
                    state["t1s"] = t1s

                def bn_mm():
                    _, r_bneg = s[f"rows_{key}"]
                    Bn = bcp.tile([128, T], f32, tag="bcast", name=f"Bn_{ssfx}")
                    nc.gpsimd.partition_broadcast(Bn[:], r_bneg[:])
                    state["Bn"] = Bn

                def t2_final():
                    outs = []
                    for m in range(MD):
                        t2 = state["t1s"][m]
                        nc.vector.tensor_tensor(t2[:], t2[:], state["Bn"][:],
                                                op=Alu.add)
                        o = out_pool.tile([128, T], out_dtype, tag=out_tag,
                                          name=f"{out_tag}_{ssfx}_{m}")
                        eng = nc.gpsimd if m >= 2 else nc.vector
                        eng.tensor_scalar(o[:], t2[:], G[m][:], BE[m][:],
                                          op0=Alu.mult, op1=Alu.add)
                        outs.append(o)
                        if dma_m is not None:
                            dma_m(o, m)
                    s[out_key] = outs

                return [r_mm, t1_ops, bn_mm, t2_final]

            def s1_mh(t, fillers=()):
                fillers = list(fillers)
                s = ST[t]
                if t + 1 < NT:
                    ST[t + 1]["x"] = x_load(t + 1)
                xt = s["x"]
                MH = []
                for m in range(MD):
                    ps = mmps.tile([128, T], f32, tag="mm", name=f"mp_{t}_{m}")
                    for k in range(KD):
                        nc.tensor.matmul(ps[:],
                                         WQNO[k][:, m * 128:(m + 1) * 128],
                                         xt[k][:], start=(k == 0),
                                         stop=(k == KD - 1))
                    mh = mhp.tile([128, T], f32r, tag="mh", bufs=5,
                                  name=f"mh_{t}_{m}")
                    nc.scalar.activation(mh[:], ps[:], AF.Identity,
                                         bias=MHB[m][:])
                    MH.append(mh)
                    if fillers:
                        fillers.pop(0)()
                s["MH"] = MH
                for f in fillers:
                    f()

            def s4_z1z2(t, fillers=()):
                fillers = list(fillers)
                s = ST[t]
                s["Z1"] = [None] * FM
                s["zps"] = [z2ps.tile([128, T], f32, tag="z2",
                                      name=f"z2_{t}_{m}") for m in range(MD)]

                def emit_z1(fm):
                    ps = mmps.tile([128, T], f32, tag="mm", name=f"z1p_{t}_{fm}")
                    for k in range(KD):
                        nc.tensor.matmul(ps[:], W1[k][:, fm * 128:(fm + 1) * 128],
                                         s["LN1"][k][:], start=(k == 0),
                                         stop=(k == KD - 1))
                    z1 = z1p.tile([128, T], f32r, tag="z1", name=f"z1_{t}_{fm}")
                    nc.scalar.activation(z1[:], ps[:], AF.Relu,
                                         bias=COLS["b1"][fm][:])
                    s["Z1"][fm] = z1

                def emit_z2(fk):
                    w2t = w2p.tile([128, D], f32r, tag="w2", name=f"w2_{t}_{fk}")
                    nc.sync.dma_start(
                        w2t[:],
                        w2_d.ap()[fk * 128:(fk + 1) * 128, :].bitcast(f32r))
                    for m in range(MD):
                        nc.tensor.matmul(s["zps"][m][:],
                                         w2t[:, m * 128:(m + 1) * 128],
                                         s["Z1"][fk][:], start=(fk == 0),
                                         stop=(fk == FM - 1))

                for fm in range(FM):
                    emit_z1(fm)
                    if fillers:
                        fillers.pop(0)()
                    if fm >= 1:
                        emit_z2(fm - 1)
                        if fillers:
                            fillers.pop(0)()
                s["emit_z2_last"] = lambda: emit_z2(FM - 1)
                for f in fillers:
                    f()

            def s6_resid(t):
                s = ST[t]
                s["emit_z2_last"]()
                SR = []
                for m in range(MD):
                    s0 = scp.tile([128, T], f32, tag="s0", name=f"s0_{t}_{m}")
                    nc.scalar.activation(s0[:], s["zps"][m][:], AF.Identity,
                                         bias=COLS["b2"][m][:])
                    sr = mhp.tile([128, T], f32r, tag="sr", bufs=8,
                                  name=f"sr_{t}_{m}")
                    nc.vector.tensor_tensor(sr[:], s0[:],
                                            s["LN1"][m][:].bitcast(f32),
                                            op=Alu.add)
                    SR.append(sr)
                s["SR"] = SR

            def s7_thunks(t):
                s = ST[t]

                def dma_m(o, m):
                    nc.gpsimd.dma_start(
                        out_d.ap()[m * 128:(m + 1) * 128,
                                   t * T:(t + 1) * T], o[:])

                return ln_norm_thunks(s, "SR", COLS["g2"], COLS["be2"],
                                      outp, "out", f32, "OUT",
                                      f"b{t}", dma_m=dma_m)

            # ---- pipeline schedule ----
            def wno_mm(p):
                bdt_ps = mmps.tile([128, 128], f32r, tag="mm", name=f"bdtp{p}")
                nc.tensor.transpose(bdt_ps[:], BD[p][:], ident[:])
                bdt = scp.tile([128, 128], f32r, tag="t1", name=f"bdt{p}")
                nc.vector.tensor_copy(bdt[:], bdt_ps[:])
                wno_ps = mmps.tile([128, D], f32, tag="mm", name=f"wnop{p}")
                nc.tensor.matmul(wno_ps[:], bdt[:], WO[p][:], start=True,
                                 stop=True)
                nc.vector.tensor_copy(WNO[p][:], wno_ps[:])

            for p in range(PAIRS):
                wno_mm(p)
            for k in range(KD):
                ps = mmps.tile([128, D], f32, tag="mm", name=f"wqnop{k}")
                for dm in range(MD):
                    nc.tensor.matmul(ps[:],
                                     WQT[dm][:, k * 128:(k + 1) * 128],
                                     WNO[dm][:], start=(dm == 0),
                                     stop=(dm == MD - 1))
                nc.vector.tensor_copy(WQNO[k][:], ps[:])
            for m in range(MD):
                ps = mmps.tile([128, 2], f32, tag="mm", name=f"mhbp{m}")
                for k in range(KD):
                    nc.tensor.matmul(ps[:],
                                     WNO[k][:, m * 128:(m + 1) * 128],
                                     BQR[k][:], start=(k == 0),
                                     stop=(k == KD - 1))
                nc.vector.tensor_tensor(MHB[m][:], ps[:, 0:1],
                                        COLS["bo"][m][:], op=Alu.add)

            ST[0]["x"] = x_load(0)
            s1_mh(0)
            for f in ln_stats_thunks(0, ST[0], "MH", "a0"):
                f()
            for t in range(1, NT + 2):
                tm1, tm2 = t - 1, t - 2
                F = []
                if tm1 < NT:
                    F += ln_norm_thunks(ST[tm1], "MH", COLS["g1"],
                                        COLS["be1"], lnp, "ln1", f32r,
                                        "LN1", f"a{tm1}")
                if 0 <= tm2 < NT:
                    F += ln_stats_thunks(tm2, ST[tm2], "SR", f"b{tm2}")
                    F += s7_thunks(tm2)
                fill4 = []
                if t < NT:
                    fill4 += ln_stats_thunks(t, ST[t], "MH", f"a{t}")
                if t < NT:
                    s1_mh(t, fillers=F[:4])
                    F = F[4:]
                else:
                    for f in F:
                        f()
                    F = []
                if tm1 < NT:
                    s4_z1z2(tm1, fillers=F + fill4)
                    s6_resid(tm1)
                else:
                    for f in F + fill4:
                        f()

    nc.compile()
    return nc


_NC = None


def _get_nc():
    global _NC
    if _NC is None:
        _NC = build_nc()
    return _NC


def kernel(x, Wq, bq, Wk, bk, Wv, bv, Wo, bo, W1, b1, W2, b2, g1, be1, g2, be2):
    nc = _get_nc()
    a = lambda v: np.ascontiguousarray(np.asarray(v, dtype=np.float32))
    x = a(x)
    shared = {
        "wq": a(Wq), "wk": a(Wk), "wv": a(Wv), "wo": a(Wo) * 2.0,
        "w1": a(W1), "w2": a(W2),
        "bq": a(bq), "bk": a(bk), "bv": a(bv), "bo": a(bo) * 2.0,
        "b1": a(b1), "b2": a(b2),
        "g1": a(g1), "be1": a(be1), "g2": a(g2), "be2": a(be2),
    }
    in_maps = [{"x": np.ascontiguousarray(x[b]), **shared} for b in range(B)]
    res = run_bass_kernel_spmd(nc, in_maps, list(range(B)))
    return np.stack([res.results[b]["out"] for b in range(B)], axis=0)

